# revision 1
# baseline (speedup 1.0000x reference)
"""Bilinear image interpolation (torch grid-sample style) on 8 Trainium2 NeuronCores.

Strategy (data-parallel over query points):
  - Shard the (4096, 4096) query grids x/y row-wise across 8 cores (512 rows each);
    replicate the 64 MB image on every core.
  - On-device prep per core: build an interleaved "row-pair" copy C of the image in
    DRAM:  C[r, 2c]   = image[r,   c] * scale
           C[r, 2c+1] = image[r+1, c] * scale
    so that the 4 pixels of any bilinear 2x2 patch (rows y0,y0+1 x cols x0,x0+1)
    are 16 CONTIGUOUS bytes:  C[y0, 2*x0 : 2*x0+4].
  - Per query: compute integer cell + weights with DVE/ACT vector ops, then fetch
    the patch with one indirect-DMA descriptor (one offset per SBUF partition per
    call), blend, mask out-of-bounds to zero, store.
"""

import sys

sys.path.insert(0, "/opt/trn_rl_repo")

import numpy as np

from contextlib import ExitStack

import concourse.bass as bass
import concourse.bacc as bacc
import concourse.tile as tile
from concourse import mybir
from concourse import bass_utils

f32 = mybir.dt.float32
i32 = mybir.dt.int32

H = W = 4096          # image
GH = GW = 4096        # query grid
NCORES = 8
SH = GH // NCORES     # query rows per core (512)
NQ = SH * GW          # queries per core (2_097_152)
F = 512               # queries per partition per chunk
NCHUNK = NQ // (128 * F)   # 16

# consts tensor columns
C_NEG_X0, C_NEG_Y0, C_INV_PS, C_HF, C_SCALE, C_HALF = range(6)

_CACHE = {}


def _build_program():
    nc = bacc.Bacc("TRN2")

    x_sh = nc.dram_tensor("x_sh", [SH, GW], f32, kind="ExternalInput")
    y_sh = nc.dram_tensor("y_sh", [SH, GW], f32, kind="ExternalInput")
    image = nc.dram_tensor("image", [H, W], f32, kind="ExternalInput")
    consts = nc.dram_tensor("consts", [128, 8], f32, kind="ExternalInput")
    out_sh = nc.dram_tensor("out_sh", [SH, GW], f32, kind="ExternalOutput")

    # interleaved row-pair copy (row r holds rows r and r+1 interleaved)
    C = nc.dram_tensor("Cinter", [H, 2 * W], f32, kind="Internal")
    C_pairs = C[:].rearrange("r (c t) -> (r c) t", t=2)   # [16.7M, 2] view
    im_pairs = image[:].rearrange("(a t) w -> a t w", t=2)  # [2048, 2, 4096]
    C_rows = C[:].rearrange("(a t) w -> a t w", t=2)        # [2048, 2, 8192]

    W2 = W // 2
    with tile.TileContext(nc) as tc:
        with tc.tile_pool(name="cpool", bufs=1) as cpool:
            consts_t = cpool.tile([128, 8], f32)
            nc.sync.dma_start(out=consts_t[:], in_=consts[:])
            scale_ap = consts_t[:, C_SCALE:C_SCALE + 1]

            # ---------------- prep: build C (half-width regions) ----------------
            with tc.tile_pool(name="ppool", bufs=2) as ppool:
                for r0 in range(0, H, 256):
                    a = r0 // 2  # pair-row base (128 pair rows per region)
                    for hx in range(2):
                        cs = hx * W2
                        evT = ppool.tile([128, W2], f32, tag="ev")
                        odT = ppool.tile([128, W2], f32, tag="od")
                        ev2T = ppool.tile([128, W2], f32, tag="ev2")
                        nc.sync.dma_start(out=evT[:], in_=im_pairs[a:a + 128, 0, cs:cs + W2])
                        nc.sync.dma_start(out=odT[:], in_=im_pairs[a:a + 128, 1, cs:cs + W2])
                        if a + 129 <= H // 2:
                            nc.sync.dma_start(out=ev2T[:], in_=im_pairs[a + 1:a + 129, 0, cs:cs + W2])
                        else:
                            # last region: 127 rows; partition 127 feeds C row 4095 (never read)
                            nc.sync.dma_start(out=ev2T[:127], in_=im_pairs[a + 1:a + 128, 0, cs:cs + W2])

                        Cme = ppool.tile([128, 2 * W2], f32, tag="cme")
                        Cmo = ppool.tile([128, 2 * W2], f32, tag="cmo")
                        me3 = Cme[:].rearrange("p (c t) -> p c t", t=2)
                        mo3 = Cmo[:].rearrange("p (c t) -> p c t", t=2)
                        nc.vector.tensor_scalar(out=me3[:, :, 0], in0=evT[:],
                                                scalar1=scale_ap, scalar2=None,
                                                op0=mybir.AluOpType.mult)
                        nc.scalar.activation(out=me3[:, :, 1], in_=odT[:],
                                             func=mybir.ActivationFunctionType.Identity,
                                             scale=scale_ap)
                        nc.vector.tensor_scalar(out=mo3[:, :, 0], in0=odT[:],
                                                scalar1=scale_ap, scalar2=None,
                                                op0=mybir.AluOpType.mult)
                        nc.scalar.activation(out=mo3[:, :, 1], in_=ev2T[:],
                                             func=mybir.ActivationFunctionType.Identity,
                                             scale=scale_ap)
                        nc.sync.dma_start(out=C_rows[a:a + 128, 0, 2 * cs:2 * cs + 2 * W2], in_=Cme[:])
                        nc.sync.dma_start(out=C_rows[a:a + 128, 1, 2 * cs:2 * cs + 2 * W2], in_=Cmo[:])

            # ---------------- main: chunks of 128 x F queries ----------------
            _stack = ExitStack()
            tpool = _stack.enter_context(tc.tile_pool(name="tpool", bufs=2))
            gpool = _stack.enter_context(tc.tile_pool(name="gpool", bufs=2))
            x_chunks = x_sh[:].rearrange("h w -> (h w)").rearrange("(k p f) -> k p f", p=128, f=F)
            y_chunks = y_sh[:].rearrange("h w -> (h w)").rearrange("(k p f) -> k p f", p=128, f=F)
            o_chunks = out_sh[:].rearrange("h w -> (h w)").rearrange("(k p f) -> k p f", p=128, f=F)
            A = mybir.AluOpType

            for k in range(NCHUNK):
                x_t = tpool.tile([128, F], f32, tag="x")
                y_t = tpool.tile([128, F], f32, tag="y")
                nc.sync.dma_start(out=x_t[:], in_=x_chunks[k])
                nc.sync.dma_start(out=y_t[:], in_=y_chunks[k])

                # tx = x - x0 ; ty = y - y0
                tx = tpool.tile([128, F], f32, tag="tx")
                ty = tpool.tile([128, F], f32, tag="ty")
                nc.vector.tensor_scalar(out=tx[:], in0=x_t[:],
                                        scalar1=consts_t[:, C_NEG_X0:C_NEG_X0 + 1],
                                        scalar2=None, op0=A.add)
                nc.scalar.activation(out=ty[:], in_=y_t[:],
                                     func=mybir.ActivationFunctionType.Identity,
                                     bias=consts_t[:, C_NEG_Y0:C_NEG_Y0 + 1])

                # pixel-space coords xi = tx/ps + (W-1)/2
                xi = tpool.tile([128, F], f32, tag="xi")
                yi = tpool.tile([128, F], f32, tag="yi")
                nc.vector.tensor_scalar(out=xi[:], in0=tx[:],
                                        scalar1=consts_t[:, C_INV_PS:C_INV_PS + 1],
                                        scalar2=2047.5, op0=A.mult, op1=A.add)
                nc.scalar.activation(out=yi[:], in_=ty[:],
                                     func=mybir.ActivationFunctionType.Identity,
                                     scale=consts_t[:, C_INV_PS:C_INV_PS + 1],
                                     bias=consts_t[:, C_HALF:C_HALF + 1])

                # clamp to [0, W-2]
                xc = tpool.tile([128, F], f32, tag="xc")
                yc = tpool.tile([128, F], f32, tag="yc")
                nc.vector.tensor_scalar(out=xc[:], in0=xi[:], scalar1=0.0,
                                        scalar2=float(W - 2), op0=A.max, op1=A.min)
                nc.vector.tensor_scalar(out=yc[:], in0=yi[:], scalar1=0.0,
                                        scalar2=float(H - 2), op0=A.max, op1=A.min)

                # floor via round-nearest + fixup:  f = rn(v); f -= (f > v)
                xI = tpool.tile([128, F], i32, tag="xI")
                yI = tpool.tile([128, F], i32, tag="yI")
                xf = tpool.tile([128, F], f32, tag="xf")
                yf = tpool.tile([128, F], f32, tag="yf")
                nc.vector.tensor_copy(out=xI[:], in_=xc[:])
                nc.vector.tensor_copy(out=yI[:], in_=yc[:])
                nc.vector.tensor_copy(out=xf[:], in_=xI[:])
                nc.vector.tensor_copy(out=yf[:], in_=yI[:])
                gx = tpool.tile([128, F], f32, tag="gx")
                gy = tpool.tile([128, F], f32, tag="gy")
                nc.vector.tensor_tensor(out=gx[:], in0=xf[:], in1=xc[:], op=A.is_gt)
                nc.vector.tensor_tensor(out=gy[:], in0=yf[:], in1=yc[:], op=A.is_gt)
                x0f = tpool.tile([128, F], f32, tag="x0f")
                y0f = tpool.tile([128, F], f32, tag="y0f")
                nc.vector.tensor_tensor(out=x0f[:], in0=xf[:], in1=gx[:], op=A.subtract)
                nc.vector.tensor_tensor(out=y0f[:], in0=yf[:], in1=gy[:], op=A.subtract)

                # weights
                dx0 = tpool.tile([128, F], f32, tag="dx0")
                dx1 = tpool.tile([128, F], f32, tag="dx1")
                dy0 = tpool.tile([128, F], f32, tag="dy0")
                dy1 = tpool.tile([128, F], f32, tag="dy1")
                nc.vector.tensor_tensor(out=dx0[:], in0=xi[:], in1=x0f[:], op=A.subtract)
                nc.vector.tensor_tensor(out=dy0[:], in0=yi[:], in1=y0f[:], op=A.subtract)
                nc.vector.tensor_scalar(out=dx1[:], in0=dx0[:], scalar1=-1.0,
                                        scalar2=1.0, op0=A.mult, op1=A.add)
                nc.vector.tensor_scalar(out=dy1[:], in0=dy0[:], scalar1=-1.0,
                                        scalar2=1.0, op0=A.mult, op1=A.add)

                # gather index (pair units): idx = y0*4096 + x0  (exact in f32, < 2^24)
                idxf = tpool.tile([128, F], f32, tag="idxf")
                nc.vector.scalar_tensor_tensor(out=idxf[:], in0=y0f[:], scalar=float(W),
                                               in1=x0f[:], op0=A.mult, op1=A.add)
                idxI = tpool.tile([128, F], i32, tag="idxI")
                nc.vector.tensor_copy(out=idxI[:], in_=idxf[:])

                # in-bounds mask: |tx| <= fov/2 and |ty| <= fov/2
                atx = tpool.tile([128, F], f32, tag="atx")
                aty = tpool.tile([128, F], f32, tag="aty")
                nc.scalar.activation(out=atx[:], in_=tx[:],
                                     func=mybir.ActivationFunctionType.Abs)
                nc.scalar.activation(out=aty[:], in_=ty[:],
                                     func=mybir.ActivationFunctionType.Abs)
                mx = tpool.tile([128, F], f32, tag="mx")
                my = tpool.tile([128, F], f32, tag="my")
                nc.vector.tensor_scalar(out=mx[:], in0=atx[:],
                                        scalar1=consts_t[:, C_HF:C_HF + 1],
                                        scalar2=None, op0=A.is_le)
                nc.vector.tensor_scalar(out=my[:], in0=aty[:],
                                        scalar1=consts_t[:, C_HF:C_HF + 1],
                                        scalar2=None, op0=A.is_le)
                inb = tpool.tile([128, F], f32, tag="inb")
                nc.vector.tensor_tensor(out=inb[:], in0=mx[:], in1=my[:], op=A.mult)

                # ---- gather: one 16B patch per query ----
                g = gpool.tile([128, F, 4], f32, tag="g")
                for i in range(F):
                    nc.gpsimd.indirect_dma_start(
                        out=g[:, i, :], out_offset=None, in_=C_pairs,
                        in_offset=bass.IndirectOffsetOnAxis(
                            ap=idxI[:, i:i + 1], axis=0),
                    )

                # ---- blend ----
                fa = g[:, :, 0]
                fb = g[:, :, 1]
                fc = g[:, :, 2]
                fd = g[:, :, 3]
                u = tpool.tile([128, F], f32, tag="u")
                v = tpool.tile([128, F], f32, tag="v")
                t1 = tpool.tile([128, F], f32, tag="t1")
                t2 = tpool.tile([128, F], f32, tag="t2")
                nc.vector.tensor_tensor(out=u[:], in0=fa, in1=dy1[:], op=A.mult)
                nc.vector.tensor_tensor(out=t1[:], in0=fb, in1=dy0[:], op=A.mult)
                nc.vector.tensor_tensor(out=v[:], in0=fc, in1=dy1[:], op=A.mult)
                nc.vector.tensor_tensor(out=t2[:], in0=fd, in1=dy0[:], op=A.mult)
                nc.vector.tensor_tensor(out=u[:], in0=u[:], in1=t1[:], op=A.add)
                nc.vector.tensor_tensor(out=v[:], in0=v[:], in1=t2[:], op=A.add)
                nc.vector.tensor_tensor(out=u[:], in0=u[:], in1=dx1[:], op=A.mult)
                nc.vector.tensor_tensor(out=v[:], in0=v[:], in1=dx0[:], op=A.mult)
                r = tpool.tile([128, F], f32, tag="r")
                nc.vector.tensor_tensor(out=r[:], in0=u[:], in1=v[:], op=A.add)
                nc.vector.tensor_tensor(out=r[:], in0=r[:], in1=inb[:], op=A.mult)
                nc.sync.dma_start(out=o_chunks[k], in_=r[:])
            _stack.close()

    nc.compile()
    return nc


def _get_program():
    if "nc" not in _CACHE:
        _CACHE["nc"] = _build_program()
    return _CACHE["nc"]


def _make_in_maps(x, y, x0, y0, image, pixelscale, scale):
    x = np.asarray(x, np.float32)
    y = np.asarray(y, np.float32)
    image = np.ascontiguousarray(np.asarray(image, np.float32))
    ps = np.float32(pixelscale)
    fov = ps * np.float32(W)          # f32, matches reference fov computation
    hf = np.float32(0.5) * fov        # exact scaling
    consts = np.zeros((128, 8), np.float32)
    consts[:, C_NEG_X0] = -np.float32(x0)
    consts[:, C_NEG_Y0] = -np.float32(y0)
    consts[:, C_INV_PS] = np.float32(1.0) / ps
    consts[:, C_HF] = hf
    consts[:, C_SCALE] = np.float32(scale)
    consts[:, C_HALF] = np.float32(2047.5)

    in_maps = []
    for c in range(NCORES):
        in_maps.append({
            "x_sh": np.ascontiguousarray(x[c * SH:(c + 1) * SH]),
            "y_sh": np.ascontiguousarray(y[c * SH:(c + 1) * SH]),
            "image": image,
            "consts": consts,
        })
    return in_maps


def kernel(x, y, x0, y0, image, pixelscale, scale, _trace=False):
    nc = _get_program()
    in_maps = _make_in_maps(x, y, x0, y0, image, pixelscale, scale)
    res = bass_utils.run_bass_kernel_spmd(
        nc, in_maps, core_ids=list(range(NCORES)), trace=_trace)
    out = np.concatenate([r["out_sh"] for r in res.results], axis=0)
    if _trace:
        kernel.last_exec_time_ns = res.exec_time_ns
    return out



# revision 2
# speedup vs baseline: 13.1089x; 13.1089x over previous
"""Bilinear image interpolation (torch grid-sample style) on 8 Trainium2 NeuronCores.

Strategy (data-parallel over query points):
  - Shard the (4096, 4096) query grids x/y row-wise across 8 cores (512 rows each);
    replicate the 64 MB image on every core.
  - On-device prep per core: build an interleaved "row-pair" copy C of the image in
    DRAM:  C[r, 2c]   = image[r,   c] * scale
           C[r, 2c+1] = image[r+1, c] * scale
    so that the 4 pixels of any bilinear 2x2 patch (rows y0,y0+1 x cols x0,x0+1)
    are 16 CONTIGUOUS bytes:  C[y0, 2*x0 : 2*x0+4].
  - Per query: compute integer cell + weights with DVE/ACT vector ops, then fetch
    the patch with one indirect-DMA descriptor (one offset per SBUF partition per
    call), blend, mask out-of-bounds to zero, store.
"""

import sys

sys.path.insert(0, "/opt/trn_rl_repo")

import numpy as np

from contextlib import ExitStack

import concourse.bass as bass
import concourse.bacc as bacc
import concourse.tile as tile
from concourse import mybir
from concourse import bass_utils

f32 = mybir.dt.float32
i32 = mybir.dt.int32

H = W = 4096          # image
GH = GW = 4096        # query grid
NCORES = 8
SH = GH // NCORES     # query rows per core (512)
NQ = SH * GW          # queries per core (2_097_152)
F = 512               # queries per partition per chunk
NCHUNK = NQ // (128 * F)   # 16

# consts tensor columns
C_NEG_X0, C_NEG_Y0, C_INV_PS, C_HF, C_SCALE, C_HALF = range(6)

_CACHE = {}


def _build_program():
    nc = bacc.Bacc("TRN2")

    x_sh = nc.dram_tensor("x_sh", [SH, GW], f32, kind="ExternalInput")
    y_sh = nc.dram_tensor("y_sh", [SH, GW], f32, kind="ExternalInput")
    image = nc.dram_tensor("image", [H, W], f32, kind="ExternalInput")
    consts = nc.dram_tensor("consts", [128, 8], f32, kind="ExternalInput")
    out_sh = nc.dram_tensor("out_sh", [SH, GW], f32, kind="ExternalOutput")

    # interleaved row-pair copy (row r holds rows r and r+1 interleaved)
    C = nc.dram_tensor("Cinter", [H, 2 * W], f32, kind="Internal")
    C_pairs = C[:].rearrange("r (c t) -> (r c) t", t=2)   # [16.7M, 2] view
    im_pairs = image[:].rearrange("(a t) w -> a t w", t=2)  # [2048, 2, 4096]
    C_rows = C[:].rearrange("(a t) w -> a t w", t=2)        # [2048, 2, 8192]

    W2 = W // 2
    with tile.TileContext(nc) as tc:
        with tc.tile_pool(name="cpool", bufs=1) as cpool:
            consts_t = cpool.tile([128, 8], f32)
            nc.sync.dma_start(out=consts_t[:], in_=consts[:])
            scale_ap = consts_t[:, C_SCALE:C_SCALE + 1]

            # ---------------- prep: build C (half-width regions) ----------------
            with tc.tile_pool(name="ppool", bufs=2) as ppool:
                for r0 in range(0, H, 256):
                    a = r0 // 2  # pair-row base (128 pair rows per region)
                    for hx in range(2):
                        cs = hx * W2
                        evT = ppool.tile([128, W2], f32, tag="ev")
                        odT = ppool.tile([128, W2], f32, tag="od")
                        ev2T = ppool.tile([128, W2], f32, tag="ev2")
                        nc.sync.dma_start(out=evT[:], in_=im_pairs[a:a + 128, 0, cs:cs + W2])
                        nc.sync.dma_start(out=odT[:], in_=im_pairs[a:a + 128, 1, cs:cs + W2])
                        if a + 129 <= H // 2:
                            nc.sync.dma_start(out=ev2T[:], in_=im_pairs[a + 1:a + 129, 0, cs:cs + W2])
                        else:
                            # last region: 127 rows; partition 127 feeds C row 4095 (never read)
                            nc.sync.dma_start(out=ev2T[:127], in_=im_pairs[a + 1:a + 128, 0, cs:cs + W2])

                        Cme = ppool.tile([128, 2 * W2], f32, tag="cme")
                        Cmo = ppool.tile([128, 2 * W2], f32, tag="cmo")
                        me3 = Cme[:].rearrange("p (c t) -> p c t", t=2)
                        mo3 = Cmo[:].rearrange("p (c t) -> p c t", t=2)
                        nc.vector.tensor_scalar(out=me3[:, :, 0], in0=evT[:],
                                                scalar1=scale_ap, scalar2=None,
                                                op0=mybir.AluOpType.mult)
                        nc.scalar.activation(out=me3[:, :, 1], in_=odT[:],
                                             func=mybir.ActivationFunctionType.Identity,
                                             scale=scale_ap)
                        nc.vector.tensor_scalar(out=mo3[:, :, 0], in0=odT[:],
                                                scalar1=scale_ap, scalar2=None,
                                                op0=mybir.AluOpType.mult)
                        nc.scalar.activation(out=mo3[:, :, 1], in_=ev2T[:],
                                             func=mybir.ActivationFunctionType.Identity,
                                             scale=scale_ap)
                        nc.sync.dma_start(out=C_rows[a:a + 128, 0, 2 * cs:2 * cs + 2 * W2], in_=Cme[:])
                        nc.sync.dma_start(out=C_rows[a:a + 128, 1, 2 * cs:2 * cs + 2 * W2], in_=Cmo[:])

            # ---------------- main: chunks of 128 x F queries ----------------
            _stack = ExitStack()
            tpool = _stack.enter_context(tc.tile_pool(name="tpool", bufs=2))
            gpool = _stack.enter_context(tc.tile_pool(name="gpool", bufs=2))
            x_chunks = x_sh[:].rearrange("h w -> (h w)").rearrange("(k p f) -> k p f", p=128, f=F)
            y_chunks = y_sh[:].rearrange("h w -> (h w)").rearrange("(k p f) -> k p f", p=128, f=F)
            o_chunks = out_sh[:].rearrange("h w -> (h w)").rearrange("(k p f) -> k p f", p=128, f=F)
            A = mybir.AluOpType

            for k in range(NCHUNK):
                x_t = tpool.tile([128, F], f32, tag="x")
                y_t = tpool.tile([128, F], f32, tag="y")
                nc.sync.dma_start(out=x_t[:], in_=x_chunks[k])
                nc.sync.dma_start(out=y_t[:], in_=y_chunks[k])

                # tx = x - x0 ; ty = y - y0
                tx = tpool.tile([128, F], f32, tag="tx")
                ty = tpool.tile([128, F], f32, tag="ty")
                nc.vector.tensor_scalar(out=tx[:], in0=x_t[:],
                                        scalar1=consts_t[:, C_NEG_X0:C_NEG_X0 + 1],
                                        scalar2=None, op0=A.add)
                nc.scalar.activation(out=ty[:], in_=y_t[:],
                                     func=mybir.ActivationFunctionType.Identity,
                                     bias=consts_t[:, C_NEG_Y0:C_NEG_Y0 + 1])

                # pixel-space coords xi = tx/ps + (W-1)/2
                xi = tpool.tile([128, F], f32, tag="xi")
                yi = tpool.tile([128, F], f32, tag="yi")
                nc.vector.tensor_scalar(out=xi[:], in0=tx[:],
                                        scalar1=consts_t[:, C_INV_PS:C_INV_PS + 1],
                                        scalar2=2047.5, op0=A.mult, op1=A.add)
                nc.scalar.activation(out=yi[:], in_=ty[:],
                                     func=mybir.ActivationFunctionType.Identity,
                                     scale=consts_t[:, C_INV_PS:C_INV_PS + 1],
                                     bias=consts_t[:, C_HALF:C_HALF + 1])

                # clamp to [0, W-2]
                xc = tpool.tile([128, F], f32, tag="xc")
                yc = tpool.tile([128, F], f32, tag="yc")
                nc.vector.tensor_scalar(out=xc[:], in0=xi[:], scalar1=0.0,
                                        scalar2=float(W - 2), op0=A.max, op1=A.min)
                nc.vector.tensor_scalar(out=yc[:], in0=yi[:], scalar1=0.0,
                                        scalar2=float(H - 2), op0=A.max, op1=A.min)

                # floor via round-nearest + fixup:  f = rn(v); f -= (f > v)
                xI = tpool.tile([128, F], i32, tag="xI")
                yI = tpool.tile([128, F], i32, tag="yI")
                xf = tpool.tile([128, F], f32, tag="xf")
                yf = tpool.tile([128, F], f32, tag="yf")
                nc.vector.tensor_copy(out=xI[:], in_=xc[:])
                nc.vector.tensor_copy(out=yI[:], in_=yc[:])
                nc.vector.tensor_copy(out=xf[:], in_=xI[:])
                nc.vector.tensor_copy(out=yf[:], in_=yI[:])
                gx = tpool.tile([128, F], f32, tag="gx")
                gy = tpool.tile([128, F], f32, tag="gy")
                nc.vector.tensor_tensor(out=gx[:], in0=xf[:], in1=xc[:], op=A.is_gt)
                nc.vector.tensor_tensor(out=gy[:], in0=yf[:], in1=yc[:], op=A.is_gt)
                x0f = tpool.tile([128, F], f32, tag="x0f")
                y0f = tpool.tile([128, F], f32, tag="y0f")
                nc.vector.tensor_tensor(out=x0f[:], in0=xf[:], in1=gx[:], op=A.subtract)
                nc.vector.tensor_tensor(out=y0f[:], in0=yf[:], in1=gy[:], op=A.subtract)

                # weights
                dx0 = tpool.tile([128, F], f32, tag="dx0")
                dx1 = tpool.tile([128, F], f32, tag="dx1")
                dy0 = tpool.tile([128, F], f32, tag="dy0")
                dy1 = tpool.tile([128, F], f32, tag="dy1")
                nc.vector.tensor_tensor(out=dx0[:], in0=xi[:], in1=x0f[:], op=A.subtract)
                nc.vector.tensor_tensor(out=dy0[:], in0=yi[:], in1=y0f[:], op=A.subtract)
                nc.vector.tensor_scalar(out=dx1[:], in0=dx0[:], scalar1=-1.0,
                                        scalar2=1.0, op0=A.mult, op1=A.add)
                nc.vector.tensor_scalar(out=dy1[:], in0=dy0[:], scalar1=-1.0,
                                        scalar2=1.0, op0=A.mult, op1=A.add)

                # gather index (pair units): idx = y0*4096 + x0  (exact in f32, < 2^24)
                idxf = tpool.tile([128, F], f32, tag="idxf")
                nc.vector.scalar_tensor_tensor(out=idxf[:], in0=y0f[:], scalar=float(W),
                                               in1=x0f[:], op0=A.mult, op1=A.add)
                idxI = tpool.tile([128, F], i32, tag="idxI")
                nc.vector.tensor_copy(out=idxI[:], in_=idxf[:])

                # in-bounds mask: |tx| <= fov/2 and |ty| <= fov/2
                atx = tpool.tile([128, F], f32, tag="atx")
                aty = tpool.tile([128, F], f32, tag="aty")
                nc.scalar.activation(out=atx[:], in_=tx[:],
                                     func=mybir.ActivationFunctionType.Abs)
                nc.scalar.activation(out=aty[:], in_=ty[:],
                                     func=mybir.ActivationFunctionType.Abs)
                mx = tpool.tile([128, F], f32, tag="mx")
                my = tpool.tile([128, F], f32, tag="my")
                nc.vector.tensor_scalar(out=mx[:], in0=atx[:],
                                        scalar1=consts_t[:, C_HF:C_HF + 1],
                                        scalar2=None, op0=A.is_le)
                nc.vector.tensor_scalar(out=my[:], in0=aty[:],
                                        scalar1=consts_t[:, C_HF:C_HF + 1],
                                        scalar2=None, op0=A.is_le)
                inb = tpool.tile([128, F], f32, tag="inb")
                nc.vector.tensor_tensor(out=inb[:], in0=mx[:], in1=my[:], op=A.mult)

                # ---- gather: one 16B patch per query, one DGE call per chunk ----
                g = gpool.tile([128, F, 4], f32, tag="g")
                nc.gpsimd.indirect_dma_start(
                    out=g[:], out_offset=None, in_=C_pairs,
                    in_offset=bass.IndirectOffsetOnAxis(ap=idxI[:], axis=0),
                )

                # ---- blend ----
                fa = g[:, :, 0]
                fb = g[:, :, 1]
                fc = g[:, :, 2]
                fd = g[:, :, 3]
                u = tpool.tile([128, F], f32, tag="u")
                v = tpool.tile([128, F], f32, tag="v")
                t1 = tpool.tile([128, F], f32, tag="t1")
                t2 = tpool.tile([128, F], f32, tag="t2")
                nc.vector.tensor_tensor(out=u[:], in0=fa, in1=dy1[:], op=A.mult)
                nc.vector.tensor_tensor(out=t1[:], in0=fb, in1=dy0[:], op=A.mult)
                nc.vector.tensor_tensor(out=v[:], in0=fc, in1=dy1[:], op=A.mult)
                nc.vector.tensor_tensor(out=t2[:], in0=fd, in1=dy0[:], op=A.mult)
                nc.vector.tensor_tensor(out=u[:], in0=u[:], in1=t1[:], op=A.add)
                nc.vector.tensor_tensor(out=v[:], in0=v[:], in1=t2[:], op=A.add)
                nc.vector.tensor_tensor(out=u[:], in0=u[:], in1=dx1[:], op=A.mult)
                nc.vector.tensor_tensor(out=v[:], in0=v[:], in1=dx0[:], op=A.mult)
                r = tpool.tile([128, F], f32, tag="r")
                nc.vector.tensor_tensor(out=r[:], in0=u[:], in1=v[:], op=A.add)
                nc.vector.tensor_tensor(out=r[:], in0=r[:], in1=inb[:], op=A.mult)
                nc.sync.dma_start(out=o_chunks[k], in_=r[:])
            _stack.close()

    nc.compile()
    return nc


def _get_program():
    if "nc" not in _CACHE:
        _CACHE["nc"] = _build_program()
    return _CACHE["nc"]


def _make_in_maps(x, y, x0, y0, image, pixelscale, scale):
    x = np.asarray(x, np.float32)
    y = np.asarray(y, np.float32)
    image = np.ascontiguousarray(np.asarray(image, np.float32))
    ps = np.float32(pixelscale)
    fov = ps * np.float32(W)          # f32, matches reference fov computation
    hf = np.float32(0.5) * fov        # exact scaling
    consts = np.zeros((128, 8), np.float32)
    consts[:, C_NEG_X0] = -np.float32(x0)
    consts[:, C_NEG_Y0] = -np.float32(y0)
    consts[:, C_INV_PS] = np.float32(1.0) / ps
    consts[:, C_HF] = hf
    consts[:, C_SCALE] = np.float32(scale)
    consts[:, C_HALF] = np.float32(2047.5)

    in_maps = []
    for c in range(NCORES):
        in_maps.append({
            "x_sh": np.ascontiguousarray(x[c * SH:(c + 1) * SH]),
            "y_sh": np.ascontiguousarray(y[c * SH:(c + 1) * SH]),
            "image": image,
            "consts": consts,
        })
    return in_maps


def kernel(x, y, x0, y0, image, pixelscale, scale, _trace=False):
    nc = _get_program()
    in_maps = _make_in_maps(x, y, x0, y0, image, pixelscale, scale)
    res = bass_utils.run_bass_kernel_spmd(
        nc, in_maps, core_ids=list(range(NCORES)), trace=_trace)
    out = np.concatenate([r["out_sh"] for r in res.results], axis=0)
    if _trace:
        kernel.last_exec_time_ns = res.exec_time_ns
    return out

